# revision 34
# baseline (speedup 1.0000x reference)
"""AttentionalSplatting TRN2 kernel.

Sharding: data-parallel over T (16 timesteps) across 8 cores, 2 timesteps per
core. Weights replicated (baked into the NEFF, see below). All heavy
arithmetic runs on device; the host does layout permutation, wire
quantization, and the tiny spatial-bias row construction.

Wall time here is dominated by the axon tunnel (~80-160 MB/s up, ~60 MB/s
down, strictly serial), so the wire format is aggressively narrowed:
  - fpe/tpe (Q/K-side activations) ship as int8, quantized per token row.
    The per-row scales are never shipped or undone: QK-LayerNorm is exactly
    invariant to per-row affine rescaling, so they cancel on device.
  - utt (V-side) ships as fp16; weights are fp16 Const tensors embedded in
    the NEFF via inline_tensor (never cross the wire). kernel() hashes the
    weights and rebuilds if they ever change, so arbitrary inputs stay
    correct.
  - The output ships back as uint8 [HW, 512+4]: per output row
    q = trunc(x * 126.99/rowmax + 128.5), with the fp32 rowmax bitcast into
    the 4 trailing bytes. Host dequantizes to fp32.
  - jax's persistent compilation cache is enabled because run_bass_via_pjrt
    rebuilds its jit closure per call; without it every call recompiles the
    NEFF executable (~0.6 s).

Per-timestep device pipeline (fp16 matmuls, fp32 PSUM/softmax/LN stats):
  Q = fpe @ WqT   (natural [q, dk] layout, PSUM)    -> LN stats -> apply
  K = tpe @ WkT   likewise; V = utt @ WvT -> V-hat [k, 8, 65] with ones col
  Qln/Kln PE-transposed to [dk, q]; gamma_q*gamma_k/8 folded into K side.
  scoresT[k,q] per head = Kh^T.T @ Qh^T  (+ spatial bias via a rank-6 f32r
  matmul on host-built position rows: -2*d2 = 4 tr.fp - 2|tr|^2 - 2|fp|^2,
  with the squared-norm terms hi/lo split for exact cancellation).
  The query-side rows also carry C(t,q) = 2*min_k d^2 - 4, a host-computed
  per-query logit shift (softmax is shift-invariant) that pins the max exp
  argument near [-4-|qk|, 0] so fp16 exp neither overflows nor underflows
  to an all-zero denominator for any track layout; the denominator also
  gets +1e-30 before reciprocal as a belt-and-braces NaN guard.
  U_h[q, 65] = expS^T.T @ Vhat_h  (col 64 = softmax denom) -> recip -> scale
  out = U @ WoT via PE transpose of U, accumulate, quantize, DMA out.
"""

import hashlib
import os
from contextlib import ExitStack

import numpy as np
import ml_dtypes

# run_bass_via_pjrt builds a fresh jit closure per call, so the in-memory jit
# cache always misses and every kernel() invocation would recompile the NEFF
# executable (~0.6 s). The persistent cache keys on HLO bytes instead, turning
# those recompiles into a disk hit.
import jax

try:
    jax.config.update("jax_compilation_cache_dir", "/tmp/.attn_splat_jax_cache")
    jax.config.update("jax_persistent_cache_min_compile_time_secs", 0)
    jax.config.update("jax_persistent_cache_min_entry_size_bytes", 0)
except Exception:
    pass

import concourse.mybir as mybir
import concourse.tile as tile
from concourse import bacc, bass_utils
from concourse.masks import make_identity

F32 = mybir.dt.float32
BF16 = mybir.dt.bfloat16
F16 = mybir.dt.float16
BF16_NP = ml_dtypes.bfloat16

T_PER_CORE = 2
N_CORES = 8
HW = 1024  # queries
M = 256    # tracks/keys
D = 512    # d_model = d_k
H = 8
HD = 64
EPS = 1e-6

LAST_RESULT = None


QSCALE = 126.99  # uint8 quant scale; .99 guards the 255.5 round-up edge


def _build_bass(weights):
    nc = bacc.Bacc("TRN2", target_bir_lowering=False)

    # Per-core DRAM inputs in [D, tokens] layout; the tiny bias side-band is
    # fp32 for the f32r bias matmul. Weights and gammas are NEFF-embedded
    # constants — they never cross the wire.
    fpeT = nc.dram_tensor("fpeT", [T_PER_CORE, D, HW], mybir.dt.int8, kind="ExternalInput").ap()
    tpeT = nc.dram_tensor("tpeT", [T_PER_CORE, D, M], mybir.dt.int8, kind="ExternalInput").ap()
    uttT = nc.dram_tensor("uttT", [T_PER_CORE, D, M], F16, kind="ExternalInput").ap()
    extk = nc.dram_tensor("extk", [T_PER_CORE, 6, M], F32, kind="ExternalInput").ap()
    extq = nc.dram_tensor("extq", [T_PER_CORE, 6, HW], F32, kind="ExternalInput").ap()
    wqT = nc.inline_tensor(weights["wqT"], "wqT").ap()
    wkT = nc.inline_tensor(weights["wkT"], "wkT").ap()
    wvT = nc.inline_tensor(weights["wvT"], "wvT").ap()
    woT = nc.inline_tensor(weights["woT"], "woT").ap()
    gx = nc.inline_tensor(weights["gx"], "gx").ap()
    out = nc.dram_tensor("out", [T_PER_CORE, HW, D + 4], mybir.dt.uint8, kind="ExternalOutput").ap()

    with tile.TileContext(nc) as tc, ExitStack() as ctx:
        singles = ctx.enter_context(tc.tile_pool(name="singles", bufs=1))
        ins = ctx.enter_context(tc.tile_pool(name="ins", bufs=2))
        work = ctx.enter_context(tc.tile_pool(name="work", bufs=2))
        work1 = ctx.enter_context(tc.tile_pool(name="work1", bufs=1))
        small = ctx.enter_context(tc.tile_pool(name="small", bufs=2))
        exps = ctx.enter_context(tc.tile_pool(name="exps", bufs=16))
        outs = ctx.enter_context(tc.tile_pool(name="outs", bufs=2))
        pA = ctx.enter_context(tc.tile_pool(name="pA", bufs=2, space="PSUM"))
        pS = ctx.enter_context(tc.tile_pool(name="pS", bufs=2, space="PSUM"))

        # ---- one-time constants ----
        ident = singles.tile([128, 128], F16)
        make_identity(nc, ident)

        w_sb = {}
        for name, ap in (("wq", wqT), ("wk", wkT), ("wv", wvT), ("wo", woT)):
            wt = singles.tile([128, 4, D], F16, tag=name)
            nc.gpsimd.dma_start(out=wt, in_=ap.rearrange("(c p) n -> p c n", p=128))
            w_sb[name] = wt

        eps_sb = singles.tile([128, 1], F32, tag="eps")
        nc.vector.memset(eps_sb, EPS)
        c1285 = singles.tile([128, 1], F32, tag="c1285")
        nc.vector.memset(c1285, 128.5)
        ctiny = singles.tile([128, 1], F32, tag="ctiny")
        nc.vector.memset(ctiny, 1e-30)
        g_all = singles.tile([128, 4], F32, tag="g_all")
        nc.sync.dma_start(out=g_all, in_=gx.rearrange("(p c) -> p c", c=4))

        for t in range(T_PER_CORE):
            ext_k = small.tile([6, M], F32, tag="ext_k")
            nc.sync.dma_start(out=ext_k, in_=extk[t])
            ext_q = small.tile([6, HW], F32, tag="ext_q")
            nc.sync.dma_start(out=ext_q, in_=extq[t])

            # ---- load per-t activations (fpe/tpe int8 on the wire, DMA-cast
            # to fp16; the per-row quant scales cancel in LN) ----
            fpe_sb = ins.tile([128, 4, HW], F16, tag="fpe")
            nc.gpsimd.dma_start(out=fpe_sb, in_=fpeT[t].rearrange("(c p) q -> p c q", p=128))
            tpe_sb = ins.tile([128, 4, M], F16, tag="tpe")
            nc.gpsimd.dma_start(out=tpe_sb, in_=tpeT[t].rearrange("(c p) q -> p c q", p=128))
            utt_sb = ins.tile([128, 4, M], F16, tag="utt")
            nc.gpsimd.dma_start(out=utt_sb, in_=uttT[t].rearrange("(c p) q -> p c q", p=128))

            # ---- projections + LN stats ----
            q_raw = work1.tile([128, 8, D], F16, tag="q_raw")
            k_raw = work1.tile([128, 2, D], F16, tag="k_raw")
            mv_all = work.tile([128, 10, 2], F32, tag="mv")
            for i in range(8):
                ps_q = pA.tile([128, D], F32, tag="pA")
                for c in range(4):
                    nc.tensor.matmul(
                        ps_q,
                        lhsT=fpe_sb[:, c, i * 128:(i + 1) * 128],
                        rhs=w_sb["wq"][:, c, :],
                        start=(c == 0), stop=(c == 3),
                    )
                nc.vector.tensor_copy(q_raw[:, i, :], ps_q)
                st = small.tile([128, 6], F32, tag="st")
                nc.vector.bn_stats(out=st, in_=q_raw[:, i, :])
                nc.vector.bn_aggr(out=mv_all[:, i, :], in_=st)
            for a in range(2):
                ps_k = pA.tile([128, D], F32, tag="pA")
                for c in range(4):
                    nc.tensor.matmul(
                        ps_k,
                        lhsT=tpe_sb[:, c, a * 128:(a + 1) * 128],
                        rhs=w_sb["wk"][:, c, :],
                        start=(c == 0), stop=(c == 3),
                    )
                nc.vector.tensor_copy(k_raw[:, a, :], ps_k)
                st = small.tile([128, 6], F32, tag="st")
                nc.vector.bn_stats(out=st, in_=k_raw[:, a, :])
                nc.vector.bn_aggr(out=mv_all[:, 8 + a, :], in_=st)

            # V projection straight into V-hat layout [k, 8 heads, 65]
            vhat = work1.tile([128, 2, H, 65], F16, tag="vhat")
            nc.gpsimd.memset(vhat[:, :, :, 64:65], 1.0)
            for a in range(2):
                ps_v = pA.tile([128, D], F32, tag="pA")
                for c in range(4):
                    nc.tensor.matmul(
                        ps_v,
                        lhsT=utt_sb[:, c, a * 128:(a + 1) * 128],
                        rhs=w_sb["wv"][:, c, :],
                        start=(c == 0), stop=(c == 3),
                    )
                nc.vector.tensor_copy(
                    vhat[:, a, :, 0:64], ps_v.rearrange("p (h d) -> p h d", h=H)
                )

            # rstd = exp(-0.5 * ln(var + eps)) : stays in the exp table set
            rstd = work.tile([128, 10], F32, tag="rstd")
            nc.scalar.activation(out=rstd, in_=mv_all[:, :, 1], func=mybir.ActivationFunctionType.Ln, bias=eps_sb)
            nc.scalar.activation(out=rstd, in_=rstd, func=mybir.ActivationFunctionType.Exp, scale=-0.5)

            # ---- LN apply + transpose to [dk, q] ----
            q_ln = work1.tile([128, 8, D], F16, tag="q_ln")
            for i in range(8):
                nc.vector.tensor_scalar(
                    out=q_ln[:, i, :], in0=q_raw[:, i, :],
                    scalar1=mv_all[:, i, 0:1], scalar2=rstd[:, i:i + 1],
                    op0=mybir.AluOpType.subtract, op1=mybir.AluOpType.mult,
                )
            k_ln = work1.tile([128, 2, D], F16, tag="k_ln")
            for a in range(2):
                nc.vector.tensor_scalar(
                    out=k_ln[:, a, :], in0=k_raw[:, a, :],
                    scalar1=mv_all[:, 8 + a, 0:1], scalar2=rstd[:, 8 + a:9 + a],
                    op0=mybir.AluOpType.subtract, op1=mybir.AluOpType.mult,
                )

            qT = work1.tile([128, 4, HW], F16, tag="qT")
            for c in range(4):
                for half in range(2):
                    ps_tr = pA.tile([128, D], F16, tag="pT")
                    for j in range(4):
                        i = half * 4 + j
                        nc.tensor.transpose(
                            ps_tr[:, j * 128:(j + 1) * 128],
                            q_ln[:, i, c * 128:(c + 1) * 128], ident,
                        )
                    nc.vector.tensor_copy(qT[:, c, half * 512:(half + 1) * 512], ps_tr)
            kT = work1.tile([128, 4, M], F16, tag="kT")
            for c in range(4):
                ps_tr = pA.tile([128, D], F16, tag="pT")
                for a in range(2):
                    nc.tensor.transpose(
                        ps_tr[:, a * 128:(a + 1) * 128],
                        k_ln[:, a, c * 128:(c + 1) * 128], ident,
                    )
                # fold gamma_q*gamma_k/8 into the K side (per-partition here)
                nc.vector.tensor_scalar_mul(
                    out=kT[:, c, :], in0=ps_tr[:, 0:M], scalar1=g_all[:, c:c + 1]
                )

            # ---- scores + bias + exp, per (head, k-tile) ----
            exp_sb = {}
            for h in range(H):
                c, po = h // 2, (h % 2) * 64
                for a in range(2):
                    ps_s = pS.tile([128, 1024], F32, tag="pS")
                    for b in range(2):
                        sl = slice(b * 512, (b + 1) * 512)
                        nc.tensor.matmul(
                            ps_s[:, sl],
                            lhsT=kT[po:po + 64, c, a * 128:(a + 1) * 128],
                            rhs=qT[po:po + 64, c, sl],
                            start=True, stop=False,
                        )
                        nc.tensor.matmul(
                            ps_s[:, sl],
                            lhsT=ext_k[:, a * 128:(a + 1) * 128],
                            rhs=ext_q[:, sl],
                            start=False, stop=True,
                        )
                    es = exps.tile([128, HW], F16, tag="exps")
                    nc.scalar.activation(out=es, in_=ps_s, func=mybir.ActivationFunctionType.Exp)
                    exp_sb[(h, a)] = es

            # ---- AV (U natural [q, 65] per head) + normalize ----
            u_norm = work1.tile([128, 8, D], F16, tag="u_norm")
            for i in range(8):
                qsl = slice(i * 128, (i + 1) * 128)
                ps_u0 = pA.tile([128, 4, 65], F32, tag="pA")
                ps_u1 = pA.tile([128, 4, 65], F32, tag="pA")
                ps_u = [ps_u0, ps_u1]
                for h in range(H):
                    grp, slot = h // 4, h % 4
                    for a in range(2):
                        nc.tensor.matmul(
                            ps_u[grp][:, slot, :],
                            lhsT=exp_sb[(h, a)][:, qsl],
                            rhs=vhat[:, a, h, :],
                            start=(a == 0), stop=(a == 1),
                        )
                # 1/(denom + tiny): a query with every exp underflowed to zero
                # yields 0 output instead of 0*inf = NaN.
                r8 = small.tile([128, 8], F32, tag="r8")
                for grp in range(2):
                    nc.vector.tensor_scalar_add(
                        out=r8[:, grp * 4:(grp + 1) * 4], in0=ps_u[grp][:, :, 64],
                        scalar1=ctiny,
                    )
                nc.vector.reciprocal(out=r8, in_=r8)
                for h in range(H):
                    grp, slot = h // 4, h % 4
                    nc.vector.tensor_scalar_mul(
                        out=u_norm[:, i, h * 64:(h + 1) * 64],
                        in0=ps_u[grp][:, slot, 0:64],
                        scalar1=r8[:, h:h + 1],
                    )

            # ---- transpose U, output projection, store ----
            uT = work1.tile([128, 4, HW], F16, tag="uT")
            for c in range(4):
                for half in range(2):
                    ps_tr = pA.tile([128, D], F16, tag="pT")
                    for j in range(4):
                        i = half * 4 + j
                        nc.tensor.transpose(
                            ps_tr[:, j * 128:(j + 1) * 128],
                            u_norm[:, i, c * 128:(c + 1) * 128], ident,
                        )
                    nc.vector.tensor_copy(uT[:, c, half * 512:(half + 1) * 512], ps_tr)

            for i in range(8):
                ps_o = pA.tile([128, D], F32, tag="pA")
                for c in range(4):
                    nc.tensor.matmul(
                        ps_o,
                        lhsT=uT[:, c, i * 128:(i + 1) * 128],
                        rhs=w_sb["wo"][:, c, :],
                        start=(c == 0), stop=(c == 3),
                    )
                # per-row uint8 quant: q = trunc(x * QSCALE/rowmax + 128.5)
                rmax = outs.tile([128, 1], F32, tag="rmax")
                nc.vector.reduce_max(
                    out=rmax, in_=ps_o, axis=mybir.AxisListType.X,
                    apply_absolute_value=True,
                )
                inv = outs.tile([128, 1], F32, tag="inv")
                nc.scalar.activation(
                    out=inv, in_=rmax, func=mybir.ActivationFunctionType.Copy,
                    scale=1.0 / QSCALE, bias=1e-25,
                )
                nc.vector.reciprocal(out=inv, in_=inv)
                o_q = outs.tile([128, D], mybir.dt.uint8, tag="o_q")
                nc.vector.tensor_scalar(
                    out=o_q, in0=ps_o, scalar1=inv, scalar2=c1285,
                    op0=mybir.AluOpType.mult, op1=mybir.AluOpType.add,
                )
                nc.sync.dma_start(out=out[t, i * 128:(i + 1) * 128, 0:D], in_=o_q)
                nc.sync.dma_start(
                    out=out[t, i * 128:(i + 1) * 128, D:D + 4],
                    in_=rmax.bitcast(mybir.dt.uint8),
                )

    nc.compile()
    return nc


def _weights_pack(inputs):
    f32 = lambda x: np.asarray(x, dtype=np.float32)
    bf16 = lambda x: np.ascontiguousarray(np.asarray(x, dtype=np.float32)).astype(np.float16)

    # gamma_q*gamma_k/8 permuted so g_all[p, c] = g[c*128 + p]
    g = (f32(inputs["q_gamma"]).astype(np.float64)
         * f32(inputs["k_gamma"]).astype(np.float64) * 0.125).astype(np.float32)
    gx = np.ascontiguousarray(g.reshape(4, 128).T).reshape(D)

    return {
        "wqT": np.ascontiguousarray(bf16(inputs["W_q"]).T),
        "wkT": np.ascontiguousarray(bf16(inputs["W_k"]).T),
        "wvT": np.ascontiguousarray(bf16(inputs["W_v"]).T),
        "woT": np.ascontiguousarray(bf16(inputs["W_out"]).T),
        "gx": gx,
    }


def _weights_key(weights):
    h = hashlib.blake2b(digest_size=16)
    for k in sorted(weights):
        h.update(np.ascontiguousarray(weights[k]).view(np.uint8).tobytes())
    return h.digest()


def _host_prep(inputs):
    f32 = lambda x: np.asarray(x, dtype=np.float32)

    # Quantize activations to int8. fpe/tpe: per-token-row scale — LN is
    # invariant to per-row affine rescaling of Q/K, so the scales need not be
    # shipped or undone. utt: one global scale, folded into the downloaded
    # per-row output scales at dequant time. Then [T, tokens, D] -> [T, D, tokens].
    def q8_rows(x):
        x = np.asarray(x, np.float32)
        s = np.abs(x).max(axis=-1, keepdims=True)
        np.maximum(s, 1e-30, out=s)
        y = np.rint(x * (127.0 / s))
        return y.transpose(0, 2, 1).astype(np.int8)

    fpeT = q8_rows(inputs["feature_pos_embeddings"])
    tpeT = q8_rows(inputs["track_pos_embeddings"])
    uttT = np.asarray(inputs["updated_track_tokens"]).transpose(0, 2, 1).astype(np.float16)

    # Spatial-bias ext rows: sum_r extk[r, k] * extq[r, q] = -2|fp_q - tr_k|^2
    #   extk rows: [tr_x, tr_y, t2hi, t2lo, 1, 1]
    #   extq rows: [4fp_x, 4fp_y, 1, 1, f2hi, f2lo]
    # with t2 = -2|tr|^2, f2 = -2|fp|^2 computed in f64 and hi/lo split so the
    # quadratic terms cancel to fp32 accuracy in the f32r matmul.
    tr = f32(inputs["tracks"]).astype(np.float64)          # [T, M, 2]
    fp = f32(inputs["feature_positions"]).astype(np.float64)  # [HW, 2]
    T = tr.shape[0]
    t2 = -2.0 * (tr[..., 0] ** 2 + tr[..., 1] ** 2)        # [T, M]
    t2hi = t2.astype(np.float32)
    t2lo = (t2 - t2hi.astype(np.float64)).astype(np.float32)
    extk = np.empty((T, 6, M), np.float32)
    extk[:, 0] = tr[..., 0]
    extk[:, 1] = tr[..., 1]
    extk[:, 2] = t2hi
    extk[:, 3] = t2lo
    extk[:, 4:6] = 1.0

    # Per-query logit shift C = 2*min_k d^2 - 4 folded into the query-side
    # bias rows: softmax is shift-invariant, and this pins each query's max
    # exp argument near [-4 - |qk|, 0], keeping fp16 exp away from both
    # overflow and all-zero underflow for ANY track/position layout.
    nf = fp[:, 0] ** 2 + fp[:, 1] ** 2                     # [HW]
    nt = tr[..., 0] ** 2 + tr[..., 1] ** 2                 # [T, M]
    gram = np.einsum('qx,tkx->tqk', fp, tr)                # [T, HW, M]
    d2min = (nf[None, :, None] + nt[:, None, :] - 2.0 * gram).min(axis=2)
    f2t = -2.0 * nf[None, :] + 2.0 * d2min - 4.0           # [T, HW]
    f2hi = f2t.astype(np.float32)
    f2lo = (f2t - f2hi.astype(np.float64)).astype(np.float32)
    extq = np.empty((T, 6, HW), np.float32)
    extq[:, 0] = 4.0 * fp[:, 0]
    extq[:, 1] = 4.0 * fp[:, 1]
    extq[:, 2:4] = 1.0
    extq[:, 4] = f2hi
    extq[:, 5] = f2lo

    in_maps = []
    for core in range(N_CORES):
        sl = slice(core * T_PER_CORE, (core + 1) * T_PER_CORE)
        in_maps.append({
            "fpeT": fpeT[sl], "tpeT": tpeT[sl], "uttT": uttT[sl],
            "extk": extk[sl], "extq": extq[sl],
        })
    return in_maps


_NC_CACHE = None
_NC_KEY = None


def kernel(**inputs) -> np.ndarray:
    global _NC_CACHE, _NC_KEY, LAST_RESULT
    in_maps = _host_prep(inputs)

    weights = _weights_pack(inputs)
    key = _weights_key(weights)
    if _NC_CACHE is None or key != _NC_KEY:
        _NC_CACHE = _build_bass(weights)
        _NC_KEY = key
    nc = _NC_CACHE

    want_trace = bool(int(os.environ.get("KERNEL_TRACE", "0")))
    try:
        res = bass_utils.run_bass_kernel_spmd(
            nc, in_maps, core_ids=list(range(N_CORES)), trace=want_trace,
        )
    except ModuleNotFoundError:
        res = bass_utils.run_bass_kernel_spmd(
            nc, in_maps, core_ids=list(range(N_CORES)), trace=False,
        )
    LAST_RESULT = res
    raw = np.concatenate([r["out"] for r in res.results], axis=0)  # [T, HW, D+4] u8
    rmax = raw[:, :, D:D + 4].copy().view(np.float32)              # [T, HW, 1]
    q = raw[:, :, 0:D].astype(np.float32)
    q -= 128.0
    q *= rmax / QSCALE
    return q


# revision 35
# speedup vs baseline: 1.0491x; 1.0491x over previous
"""AttentionalSplatting TRN2 kernel.

Sharding: data-parallel over T (16 timesteps) across 8 cores, 2 timesteps per
core. Weights replicated (baked into the NEFF, see below). All heavy
arithmetic runs on device; the host does layout permutation, wire
quantization, and the tiny spatial-bias row construction.

Wall time here is dominated by the axon tunnel (~80-160 MB/s up, ~60 MB/s
down, strictly serial), so the wire format is aggressively narrowed:
  - fpe/tpe (Q/K-side activations) ship as int8, quantized per token row.
    The per-row scales are never shipped or undone: QK-LayerNorm is exactly
    invariant to per-row affine rescaling, so they cancel on device.
  - utt (V-side) ships as fp16; weights are fp16 Const tensors embedded in
    the NEFF via inline_tensor (never cross the wire). kernel() hashes the
    weights and rebuilds if they ever change, so arbitrary inputs stay
    correct.
  - The output ships back as uint8 [HW, 512+4]: per output row
    q = trunc(x * 126.99/rowmax + 128.5), with the fp32 rowmax bitcast into
    the 4 trailing bytes. Host dequantizes to fp32.
  - jax's persistent compilation cache is enabled because run_bass_via_pjrt
    rebuilds its jit closure per call; without it every call recompiles the
    NEFF executable (~0.6 s).

Per-timestep device pipeline (fp16 matmuls, fp32 PSUM/softmax/LN stats):
  Q = fpe @ WqT   (natural [q, dk] layout, PSUM)    -> LN stats -> apply
  K = tpe @ WkT   likewise; V = utt @ WvT -> V-hat [k, 8, 65] with ones col
  Qln/Kln PE-transposed to [dk, q]; gamma_q*gamma_k/8 folded into K side.
  scoresT[k,q] per head = Kh^T.T @ Qh^T  (+ spatial bias via a rank-6 f32r
  matmul on host-built position rows: -2*d2 = 4 tr.fp - 2|tr|^2 - 2|fp|^2,
  with the squared-norm terms hi/lo split for exact cancellation).
  The query-side rows also carry C(t,q) = 2*min_k d^2 - 4, a host-computed
  per-query logit shift (softmax is shift-invariant) that pins the max exp
  argument near [-4-|qk|, 0] so fp16 exp neither overflows nor underflows
  to an all-zero denominator for any track layout; the denominator also
  gets +1e-30 before reciprocal as a belt-and-braces NaN guard.
  U_h[q, 65] = expS^T.T @ Vhat_h  (col 64 = softmax denom) -> recip -> scale
  out = U @ WoT via PE transpose of U, accumulate, quantize, DMA out.
"""

import hashlib
import os
from contextlib import ExitStack

import numpy as np
import ml_dtypes

# run_bass_via_pjrt builds a fresh jit closure per call, so the in-memory jit
# cache always misses and every kernel() invocation would recompile the NEFF
# executable (~0.6 s). The persistent cache keys on HLO bytes instead, turning
# those recompiles into a disk hit.
import jax

try:
    jax.config.update("jax_compilation_cache_dir", "/tmp/.attn_splat_jax_cache")
    jax.config.update("jax_persistent_cache_min_compile_time_secs", 0)
    jax.config.update("jax_persistent_cache_min_entry_size_bytes", 0)
except Exception:
    pass

import concourse.mybir as mybir
import concourse.tile as tile
from concourse import bacc, bass_utils
from concourse.masks import make_identity

F32 = mybir.dt.float32
BF16 = mybir.dt.bfloat16
F16 = mybir.dt.float16
BF16_NP = ml_dtypes.bfloat16

T_PER_CORE = 2
N_CORES = 8
HW = 1024  # queries
M = 256    # tracks/keys
D = 512    # d_model = d_k
H = 8
HD = 64
EPS = 1e-6

LAST_RESULT = None


QSCALE = 126.99  # uint8 quant scale; .99 guards the 255.5 round-up edge


def _build_bass(weights):
    nc = bacc.Bacc("TRN2", target_bir_lowering=False)

    # Per-core DRAM inputs in [D, tokens] layout; the tiny bias side-band is
    # fp32 for the f32r bias matmul. Weights and gammas are NEFF-embedded
    # constants — they never cross the wire.
    fpeT = nc.dram_tensor("fpeT", [T_PER_CORE, D, HW], mybir.dt.int8, kind="ExternalInput").ap()
    tpeT = nc.dram_tensor("tpeT", [T_PER_CORE, D, M], mybir.dt.int8, kind="ExternalInput").ap()
    uttT = nc.dram_tensor("uttT", [T_PER_CORE, D, M], F16, kind="ExternalInput").ap()
    extk = nc.dram_tensor("extk", [T_PER_CORE, 6, M], F32, kind="ExternalInput").ap()
    extq = nc.dram_tensor("extq", [T_PER_CORE, 6, HW], F32, kind="ExternalInput").ap()
    wqT = nc.inline_tensor(weights["wqT"], "wqT").ap()
    wkT = nc.inline_tensor(weights["wkT"], "wkT").ap()
    wvT = nc.inline_tensor(weights["wvT"], "wvT").ap()
    woT = nc.inline_tensor(weights["woT"], "woT").ap()
    gx = nc.inline_tensor(weights["gx"], "gx").ap()
    out = nc.dram_tensor("out", [T_PER_CORE, HW, D + 4], mybir.dt.uint8, kind="ExternalOutput").ap()

    with tile.TileContext(nc) as tc, ExitStack() as ctx:
        singles = ctx.enter_context(tc.tile_pool(name="singles", bufs=1))
        ins = ctx.enter_context(tc.tile_pool(name="ins", bufs=2))
        work = ctx.enter_context(tc.tile_pool(name="work", bufs=2))
        work1 = ctx.enter_context(tc.tile_pool(name="work1", bufs=1))
        small = ctx.enter_context(tc.tile_pool(name="small", bufs=2))
        exps = ctx.enter_context(tc.tile_pool(name="exps", bufs=16))
        outs = ctx.enter_context(tc.tile_pool(name="outs", bufs=2))
        pA = ctx.enter_context(tc.tile_pool(name="pA", bufs=2, space="PSUM"))
        pS = ctx.enter_context(tc.tile_pool(name="pS", bufs=2, space="PSUM"))

        # ---- one-time constants ----
        ident = singles.tile([128, 128], F16)
        make_identity(nc, ident)

        w_sb = {}
        for name, ap in (("wq", wqT), ("wk", wkT), ("wv", wvT), ("wo", woT)):
            wt = singles.tile([128, 4, D], F16, tag=name)
            nc.gpsimd.dma_start(out=wt, in_=ap.rearrange("(c p) n -> p c n", p=128))
            w_sb[name] = wt

        eps_sb = singles.tile([128, 1], F32, tag="eps")
        nc.vector.memset(eps_sb, EPS)
        c1285 = singles.tile([128, 1], F32, tag="c1285")
        nc.vector.memset(c1285, 128.5)
        ctiny = singles.tile([128, 1], F32, tag="ctiny")
        nc.vector.memset(ctiny, 1e-30)
        g_all = singles.tile([128, 4], F32, tag="g_all")
        nc.sync.dma_start(out=g_all, in_=gx.rearrange("(p c) -> p c", c=4))

        for t in range(T_PER_CORE):
            ext_k = small.tile([6, M], F32, tag="ext_k")
            nc.sync.dma_start(out=ext_k, in_=extk[t])
            ext_q = small.tile([6, HW], F32, tag="ext_q")
            nc.sync.dma_start(out=ext_q, in_=extq[t])

            # ---- load per-t activations (fpe/tpe int8 on the wire, DMA-cast
            # to fp16; the per-row quant scales cancel in LN) ----
            fpe_sb = ins.tile([128, 4, HW], F16, tag="fpe")
            nc.gpsimd.dma_start(out=fpe_sb, in_=fpeT[t].rearrange("(c p) q -> p c q", p=128))
            tpe_sb = ins.tile([128, 4, M], F16, tag="tpe")
            nc.gpsimd.dma_start(out=tpe_sb, in_=tpeT[t].rearrange("(c p) q -> p c q", p=128))
            utt_sb = ins.tile([128, 4, M], F16, tag="utt")
            nc.gpsimd.dma_start(out=utt_sb, in_=uttT[t].rearrange("(c p) q -> p c q", p=128))

            # ---- projections + LN stats ----
            q_raw = work1.tile([128, 8, D], F16, tag="q_raw")
            k_raw = work1.tile([128, 2, D], F16, tag="k_raw")
            mv_all = work.tile([128, 10, 2], F32, tag="mv")
            for i in range(8):
                ps_q = pA.tile([128, D], F32, tag="pA")
                for c in range(4):
                    nc.tensor.matmul(
                        ps_q,
                        lhsT=fpe_sb[:, c, i * 128:(i + 1) * 128],
                        rhs=w_sb["wq"][:, c, :],
                        start=(c == 0), stop=(c == 3),
                    )
                nc.vector.tensor_copy(q_raw[:, i, :], ps_q)
                st = small.tile([128, 6], F32, tag="st")
                nc.vector.bn_stats(out=st, in_=q_raw[:, i, :])
                nc.vector.bn_aggr(out=mv_all[:, i, :], in_=st)
            for a in range(2):
                ps_k = pA.tile([128, D], F32, tag="pA")
                for c in range(4):
                    nc.tensor.matmul(
                        ps_k,
                        lhsT=tpe_sb[:, c, a * 128:(a + 1) * 128],
                        rhs=w_sb["wk"][:, c, :],
                        start=(c == 0), stop=(c == 3),
                    )
                nc.vector.tensor_copy(k_raw[:, a, :], ps_k)
                st = small.tile([128, 6], F32, tag="st")
                nc.vector.bn_stats(out=st, in_=k_raw[:, a, :])
                nc.vector.bn_aggr(out=mv_all[:, 8 + a, :], in_=st)

            # V projection straight into V-hat layout [k, 8 heads, 65]
            vhat = work1.tile([128, 2, H, 65], F16, tag="vhat")
            nc.gpsimd.memset(vhat[:, :, :, 64:65], 1.0)
            for a in range(2):
                ps_v = pA.tile([128, D], F32, tag="pA")
                for c in range(4):
                    nc.tensor.matmul(
                        ps_v,
                        lhsT=utt_sb[:, c, a * 128:(a + 1) * 128],
                        rhs=w_sb["wv"][:, c, :],
                        start=(c == 0), stop=(c == 3),
                    )
                nc.vector.tensor_copy(
                    vhat[:, a, :, 0:64], ps_v.rearrange("p (h d) -> p h d", h=H)
                )

            # rstd = exp(-0.5 * ln(var + eps)) : stays in the exp table set
            rstd = work.tile([128, 10], F32, tag="rstd")
            nc.scalar.activation(out=rstd, in_=mv_all[:, :, 1], func=mybir.ActivationFunctionType.Ln, bias=eps_sb)
            nc.scalar.activation(out=rstd, in_=rstd, func=mybir.ActivationFunctionType.Exp, scale=-0.5)

            # ---- LN apply + transpose to [dk, q] ----
            q_ln = work1.tile([128, 8, D], F16, tag="q_ln")
            for i in range(8):
                nc.vector.tensor_scalar(
                    out=q_ln[:, i, :], in0=q_raw[:, i, :],
                    scalar1=mv_all[:, i, 0:1], scalar2=rstd[:, i:i + 1],
                    op0=mybir.AluOpType.subtract, op1=mybir.AluOpType.mult,
                )
            k_ln = work1.tile([128, 2, D], F16, tag="k_ln")
            for a in range(2):
                nc.vector.tensor_scalar(
                    out=k_ln[:, a, :], in0=k_raw[:, a, :],
                    scalar1=mv_all[:, 8 + a, 0:1], scalar2=rstd[:, 8 + a:9 + a],
                    op0=mybir.AluOpType.subtract, op1=mybir.AluOpType.mult,
                )

            qT = work1.tile([128, 4, HW], F16, tag="qT")
            for c in range(4):
                for half in range(2):
                    ps_tr = pA.tile([128, D], F16, tag="pT")
                    for j in range(4):
                        i = half * 4 + j
                        nc.tensor.transpose(
                            ps_tr[:, j * 128:(j + 1) * 128],
                            q_ln[:, i, c * 128:(c + 1) * 128], ident,
                        )
                    nc.vector.tensor_copy(qT[:, c, half * 512:(half + 1) * 512], ps_tr)
            kT = work1.tile([128, 4, M], F16, tag="kT")
            for c in range(4):
                ps_tr = pA.tile([128, D], F16, tag="pT")
                for a in range(2):
                    nc.tensor.transpose(
                        ps_tr[:, a * 128:(a + 1) * 128],
                        k_ln[:, a, c * 128:(c + 1) * 128], ident,
                    )
                # fold gamma_q*gamma_k/8 into the K side (per-partition here)
                nc.vector.tensor_scalar_mul(
                    out=kT[:, c, :], in0=ps_tr[:, 0:M], scalar1=g_all[:, c:c + 1]
                )

            # ---- scores + bias + exp, per (head, k-tile) ----
            exp_sb = {}
            for h in range(H):
                c, po = h // 2, (h % 2) * 64
                for a in range(2):
                    ps_s = pS.tile([128, 1024], F32, tag="pS")
                    for b in range(2):
                        sl = slice(b * 512, (b + 1) * 512)
                        nc.tensor.matmul(
                            ps_s[:, sl],
                            lhsT=kT[po:po + 64, c, a * 128:(a + 1) * 128],
                            rhs=qT[po:po + 64, c, sl],
                            start=True, stop=False,
                        )
                        nc.tensor.matmul(
                            ps_s[:, sl],
                            lhsT=ext_k[:, a * 128:(a + 1) * 128],
                            rhs=ext_q[:, sl],
                            start=False, stop=True,
                        )
                    es = exps.tile([128, HW], F16, tag="exps")
                    nc.scalar.activation(out=es, in_=ps_s, func=mybir.ActivationFunctionType.Exp)
                    exp_sb[(h, a)] = es

            # ---- AV (U natural [q, 65] per head) + normalize ----
            u_norm = work1.tile([128, 8, D], F16, tag="u_norm")
            for i in range(8):
                qsl = slice(i * 128, (i + 1) * 128)
                ps_u0 = pA.tile([128, 4, 65], F32, tag="pA")
                ps_u1 = pA.tile([128, 4, 65], F32, tag="pA")
                ps_u = [ps_u0, ps_u1]
                for h in range(H):
                    grp, slot = h // 4, h % 4
                    for a in range(2):
                        nc.tensor.matmul(
                            ps_u[grp][:, slot, :],
                            lhsT=exp_sb[(h, a)][:, qsl],
                            rhs=vhat[:, a, h, :],
                            start=(a == 0), stop=(a == 1),
                        )
                # 1/(denom + tiny): a query with every exp underflowed to zero
                # yields 0 output instead of 0*inf = NaN.
                r8 = small.tile([128, 8], F32, tag="r8")
                for grp in range(2):
                    nc.vector.tensor_scalar_add(
                        out=r8[:, grp * 4:(grp + 1) * 4], in0=ps_u[grp][:, :, 64],
                        scalar1=ctiny,
                    )
                nc.vector.reciprocal(out=r8, in_=r8)
                for h in range(H):
                    grp, slot = h // 4, h % 4
                    nc.vector.tensor_scalar_mul(
                        out=u_norm[:, i, h * 64:(h + 1) * 64],
                        in0=ps_u[grp][:, slot, 0:64],
                        scalar1=r8[:, h:h + 1],
                    )

            # ---- transpose U, output projection, store ----
            uT = work1.tile([128, 4, HW], F16, tag="uT")
            for c in range(4):
                for half in range(2):
                    ps_tr = pA.tile([128, D], F16, tag="pT")
                    for j in range(4):
                        i = half * 4 + j
                        nc.tensor.transpose(
                            ps_tr[:, j * 128:(j + 1) * 128],
                            u_norm[:, i, c * 128:(c + 1) * 128], ident,
                        )
                    nc.vector.tensor_copy(uT[:, c, half * 512:(half + 1) * 512], ps_tr)

            for i in range(8):
                ps_o = pA.tile([128, D], F32, tag="pA")
                for c in range(4):
                    nc.tensor.matmul(
                        ps_o,
                        lhsT=uT[:, c, i * 128:(i + 1) * 128],
                        rhs=w_sb["wo"][:, c, :],
                        start=(c == 0), stop=(c == 3),
                    )
                # per-row uint8 quant: q = trunc(x * QSCALE/rowmax + 128.5)
                rmax = outs.tile([128, 1], F32, tag="rmax")
                nc.vector.reduce_max(
                    out=rmax, in_=ps_o, axis=mybir.AxisListType.X,
                    apply_absolute_value=True,
                )
                inv = outs.tile([128, 1], F32, tag="inv")
                nc.scalar.activation(
                    out=inv, in_=rmax, func=mybir.ActivationFunctionType.Copy,
                    scale=1.0 / QSCALE, bias=1e-25,
                )
                nc.vector.reciprocal(out=inv, in_=inv)
                o_q = outs.tile([128, D], mybir.dt.uint8, tag="o_q")
                nc.vector.tensor_scalar(
                    out=o_q, in0=ps_o, scalar1=inv, scalar2=c1285,
                    op0=mybir.AluOpType.mult, op1=mybir.AluOpType.add,
                )
                nc.sync.dma_start(out=out[t, i * 128:(i + 1) * 128, 0:D], in_=o_q)
                nc.sync.dma_start(
                    out=out[t, i * 128:(i + 1) * 128, D:D + 4],
                    in_=rmax.bitcast(mybir.dt.uint8),
                )

    nc.compile()
    return nc


def _weights_pack(inputs):
    f32 = lambda x: np.asarray(x, dtype=np.float32)
    bf16 = lambda x: np.ascontiguousarray(np.asarray(x, dtype=np.float32)).astype(np.float16)

    # gamma_q*gamma_k/8 permuted so g_all[p, c] = g[c*128 + p]
    g = (f32(inputs["q_gamma"]).astype(np.float64)
         * f32(inputs["k_gamma"]).astype(np.float64) * 0.125).astype(np.float32)
    gx = np.ascontiguousarray(g.reshape(4, 128).T).reshape(D)

    return {
        "wqT": np.ascontiguousarray(bf16(inputs["W_q"]).T),
        "wkT": np.ascontiguousarray(bf16(inputs["W_k"]).T),
        "wvT": np.ascontiguousarray(bf16(inputs["W_v"]).T),
        "woT": np.ascontiguousarray(bf16(inputs["W_out"]).T),
        "gx": gx,
    }


def _weights_key(weights):
    h = hashlib.blake2b(digest_size=16)
    for k in sorted(weights):
        h.update(np.ascontiguousarray(weights[k]).view(np.uint8).tobytes())
    return h.digest()


def _host_prep(inputs):
    f32 = lambda x: np.asarray(x, dtype=np.float32)

    # fpe/tpe ship int8 with a per-token-row scale — LN is invariant to
    # per-row affine rescaling of Q/K, so the scales need not be shipped or
    # undone. utt (V path, not LN'd) ships fp16. [T, tokens, D] -> [T, D, tokens].
    def q8_rows(x):
        x = np.asarray(x, np.float32)
        s = np.abs(x).max(axis=-1, keepdims=True)
        np.maximum(s, 1e-30, out=s)
        y = np.rint(x * (127.0 / s))
        return y.transpose(0, 2, 1).astype(np.int8)

    fpeT = q8_rows(inputs["feature_pos_embeddings"])
    tpeT = q8_rows(inputs["track_pos_embeddings"])
    uttT = np.asarray(inputs["updated_track_tokens"]).transpose(0, 2, 1).astype(np.float16)

    # Spatial-bias ext rows: sum_r extk[r, k] * extq[r, q] = -2|fp_q - tr_k|^2
    #   extk rows: [tr_x, tr_y, t2hi, t2lo, 1, 1]
    #   extq rows: [4fp_x, 4fp_y, 1, 1, f2hi, f2lo]
    # with t2 = -2|tr|^2, f2 = -2|fp|^2 computed in f64 and hi/lo split so the
    # quadratic terms cancel to fp32 accuracy in the f32r matmul.
    tr = f32(inputs["tracks"]).astype(np.float64)          # [T, M, 2]
    fp = f32(inputs["feature_positions"]).astype(np.float64)  # [HW, 2]
    T = tr.shape[0]
    t2 = -2.0 * (tr[..., 0] ** 2 + tr[..., 1] ** 2)        # [T, M]
    t2hi = t2.astype(np.float32)
    t2lo = (t2 - t2hi.astype(np.float64)).astype(np.float32)
    extk = np.empty((T, 6, M), np.float32)
    extk[:, 0] = tr[..., 0]
    extk[:, 1] = tr[..., 1]
    extk[:, 2] = t2hi
    extk[:, 3] = t2lo
    extk[:, 4:6] = 1.0

    # Per-query logit shift C = 2*min_k d^2 - 4 folded into the query-side
    # bias rows: softmax is shift-invariant, and this pins each query's max
    # exp argument near [-4 - |qk|, 0], keeping fp16 exp away from both
    # overflow and all-zero underflow for ANY track/position layout.
    nf = fp[:, 0] ** 2 + fp[:, 1] ** 2                     # [HW]
    nt = tr[..., 0] ** 2 + tr[..., 1] ** 2                 # [T, M]
    gram = np.einsum('qx,tkx->tqk', fp, tr)                # [T, HW, M]
    d2min = (nf[None, :, None] + nt[:, None, :] - 2.0 * gram).min(axis=2)
    f2t = -2.0 * nf[None, :] + 2.0 * d2min - 4.0           # [T, HW]
    f2hi = f2t.astype(np.float32)
    f2lo = (f2t - f2hi.astype(np.float64)).astype(np.float32)
    extq = np.empty((T, 6, HW), np.float32)
    extq[:, 0] = 4.0 * fp[:, 0]
    extq[:, 1] = 4.0 * fp[:, 1]
    extq[:, 2:4] = 1.0
    extq[:, 4] = f2hi
    extq[:, 5] = f2lo

    in_maps = []
    for core in range(N_CORES):
        sl = slice(core * T_PER_CORE, (core + 1) * T_PER_CORE)
        in_maps.append({
            "fpeT": fpeT[sl], "tpeT": tpeT[sl], "uttT": uttT[sl],
            "extk": extk[sl], "extq": extq[sl],
        })
    return in_maps


_NC_CACHE = None
_NC_KEY = None


def kernel(**inputs) -> np.ndarray:
    global _NC_CACHE, _NC_KEY, LAST_RESULT
    in_maps = _host_prep(inputs)

    weights = _weights_pack(inputs)
    key = _weights_key(weights)
    if _NC_CACHE is None or key != _NC_KEY:
        _NC_CACHE = _build_bass(weights)
        _NC_KEY = key
    nc = _NC_CACHE

    want_trace = bool(int(os.environ.get("KERNEL_TRACE", "0")))
    try:
        res = bass_utils.run_bass_kernel_spmd(
            nc, in_maps, core_ids=list(range(N_CORES)), trace=want_trace,
        )
    except ModuleNotFoundError:
        res = bass_utils.run_bass_kernel_spmd(
            nc, in_maps, core_ids=list(range(N_CORES)), trace=False,
        )
    LAST_RESULT = res
    raw = np.concatenate([r["out"] for r in res.results], axis=0)  # [T, HW, D+4] u8
    rmax = raw[:, :, D:D + 4].copy().view(np.float32)              # [T, HW, 1]
    q = raw[:, :, 0:D].astype(np.float32)
    q -= 128.0
    q *= rmax / QSCALE
    return q
